# revision 1
# baseline (speedup 1.0000x reference)
"""DecoderLSTM Trainium2 kernel — 8-core tensor-parallel over gate output units.

Reference semantics (c_0 frozen by the original loop's bug):
    a1 = tanh(h @ Wd1 + bd1)                  # [B, U]
    y  = a1 @ Wd2 + bd2                       # [B, TOKEN] (the per-step output)
    xh = [y, h]
    i,f,o = sigmoid(xh @ W_*), g = tanh(xh @ W_g)
    c = f*c_0 + i*g ; h' = o*tanh(c)

Kernel reformulation: Wd2 is folded into the gate weights on the host
(Wfused = Wd2 @ W_*[:TOKEN], bhat = b_* + bd2 @ W_*[:TOKEN]), so the gates
contract directly over [a1, h] (K = 2048) and the y computation moves off the
recurrence's critical path (it runs during the inter-core AllGather).

Distribution: 8-way tensor-parallel over gate output units (128 units per gate
per core, laid out i|f|o|g in a 512-wide slab). Each step ends with an
AllGather of the transposed h' slices, which lands directly in h^T layout.

All matmuls use stationary = transposed activations [K=128, B=128] and
moving = weights [K=128, N=512] in fp16 — full-rate (1 cycle/row) streaming
with fp32 PSUM accumulation.
"""
import numpy as np

import concourse.bacc as bacc
import concourse.tile as tile
import concourse.mybir as mybir
from concourse.bass_utils import run_bass_kernel_spmd

N_CORES = 8
B = 128
UNITS = 1024
TOKEN = 512
T_FULL = 256
KC = UNITS // 128
F32 = mybir.dt.float32
AFT = mybir.ActivationFunctionType

WDT_NAME = "float16"   # dtype of weights / stationary activations / exchange


def build(T: int, wdt_name: str = WDT_NAME):
    WDT = getattr(mybir.dt, wdt_name)
    nc = bacc.Bacc("TRN2", target_bir_lowering=False, debug=False,
                   num_devices=N_CORES)

    def din(name, shape, dt=F32):
        return nc.dram_tensor(name, list(shape), dt, kind="ExternalInput").ap()

    h0T_t = din("h0T", (UNITS, B), WDT)
    c0s_t = din("c0s", (B, 128))
    wd1_t = din("wd1", (UNITS, UNITS), WDT)
    wd2_t = din("wd2", (UNITS, TOKEN), WDT)
    wfu_t = din("wfu", (UNITS, 512), WDT)
    wh_t = din("wh", (UNITS, 512), WDT)
    bd1_t = din("bd1row", (1, UNITS), WDT)
    bd2_t = din("bd2row", (1, TOKEN), WDT)
    bg_t = din("bgrow", (1, 512), WDT)
    eye_t = din("eye", (128, 128))

    ys_t = nc.dram_tensor("ys", [B, T, TOKEN], F32, kind="ExternalOutput").ap()

    with tile.TileContext(nc) as tc:
        with tc.tile_pool(name="const", bufs=1) as const, \
             tc.tile_pool(name="state", bufs=2) as state, \
             tc.tile_pool(name="act", bufs=2) as act, \
             tc.tile_pool(name="ps_a1", bufs=1, space="PSUM") as ps_a1, \
             tc.tile_pool(name="ps_g", bufs=2, space="PSUM") as ps_g, \
             tc.tile_pool(name="ps_y", bufs=2, space="PSUM") as ps_y, \
             tc.tile_pool(name="ps_tr", bufs=2, space="PSUM") as ps_tr, \
             tc.tile_pool(name="dram", bufs=2, space="DRAM") as dram:

            def load_w(name, dram_ap, kdim, ndim):
                r = const.tile([128, kdim, ndim], WDT, tag=name, name=name)
                nc.sync.dma_start(r[:], dram_ap.rearrange("(c p) n -> p c n", p=128))
                return r

            wd1_r = load_w("wd1s", wd1_t, KC, UNITS)
            wd2_r = load_w("wd2s", wd2_t, KC, TOKEN)
            wfu_r = load_w("wfus", wfu_t, KC, 512)
            wh_r = load_w("whs", wh_t, KC, 512)

            def load_row(name, dram_ap, n):
                r = const.tile([1, n], WDT, tag=name, name=name)
                nc.sync.dma_start(r[:], dram_ap)
                return r

            bd1_r = load_row("bd1s", bd1_t, UNITS)
            bd2_r = load_row("bd2s", bd2_t, TOKEN)
            bg_r = load_row("bgs", bg_t, 512)

            ones_f = const.tile([1, 128], F32, tag="ones_f")
            nc.vector.memset(ones_f[:], 1.0)
            ones_r = const.tile([1, 128], WDT, tag="ones_r")
            nc.vector.tensor_copy(ones_r[:], ones_f[:])

            eye_sb = const.tile([128, 128], F32, tag="eye")
            nc.sync.dma_start(eye_sb[:], eye_t[:])

            c0_sb = const.tile([B, 128], F32, tag="c0")
            nc.sync.dma_start(c0_sb[:], c0s_t[:])

            # h state: 4 pair-tiles of 2 K-chunks each; chunk k lives at
            # hTp[k//2][:, k%2, :]. Reloads spread over 3 engine queues.
            dma_engines = [nc.sync, nc.gpsimd, nc.scalar, nc.sync]
            hTp = [state.tile([128, 2, B], WDT, tag=f"hTp{p}", name=f"hTp{p}")
                   for p in range(4)]
            for p in range(4):
                dma_engines[p].dma_start(
                    hTp[p][:],
                    h0T_t.rearrange("(c p) b -> p c b", p=128)[:, 2 * p:2 * p + 2, :])

            def hT(k):
                return hTp[k // 2][:, k % 2, :]

            for t in range(T):
                a1_ps = ps_a1.tile([B, UNITS], F32, tag="a1")
                g_ps = ps_g.tile([B, 512], F32, tag="g")

                # Dense1 + bias
                for half in range(2):
                    o0 = half * 512
                    nc.tensor.matmul(
                        a1_ps[:, o0:o0 + 512], ones_r[:], bd1_r[:, o0:o0 + 512],
                        start=True, stop=False,
                    )
                    for k in range(KC):
                        nc.tensor.matmul(
                            a1_ps[:, o0:o0 + 512], hT(k),
                            wd1_r[:, k, o0:o0 + 512],
                            start=False, stop=(k == KC - 1),
                        )

                # gates h-part (bias first so nothing trails the last chunk)
                nc.tensor.matmul(g_ps[:], ones_r[:], bg_r[:], start=True, stop=False)
                for k in range(KC):
                    nc.tensor.matmul(
                        g_ps[:], hT(k), wh_r[:, k, :],
                        start=False, stop=False,
                    )

                # tanh(a1) -> SBUF fp32, 2 halves; PE transpose -> a1T (WDT)
                a1_sb = act.tile([B, UNITS], F32, tag="a1_sb")
                a1T = act.tile([128, KC, B], WDT, tag="a1T")
                for grp in range(2):
                    o0 = grp * 512
                    nc.scalar.activation(a1_sb[:, o0:o0 + 512],
                                         a1_ps[:, o0:o0 + 512], AFT.Tanh)
                    tr = ps_tr.tile([128, 4, B], F32, tag="tr")
                    for i in range(4):
                        c = grp * 4 + i
                        nc.tensor.transpose(tr[:, i, :],
                                            a1_sb[:, 128 * c:128 * (c + 1)],
                                            eye_sb[:])
                    nc.vector.tensor_copy(a1T[:, 4 * grp:4 * (grp + 1), :], tr[:])

                # gates a1-part + bias
                for k in range(KC):
                    nc.tensor.matmul(
                        g_ps[:], a1T[:, k, :], wfu_r[:, k, :],
                        start=False, stop=(k == KC - 1),
                    )

                # nonlinearities: i|f|o sigmoid, g tanh
                gact = act.tile([B, 512], F32, tag="gact")
                nc.scalar.activation(gact[:, 0:384], g_ps[:, 0:384], AFT.Sigmoid)
                nc.scalar.activation(gact[:, 384:512], g_ps[:, 384:512], AFT.Tanh)

                # c = f*c0 + i*g ; h' = o*tanh(c)
                ig = act.tile([B, 128], F32, tag="ig")
                nc.vector.tensor_mul(ig[:], gact[:, 0:128], gact[:, 384:512])
                cn = act.tile([B, 128], F32, tag="cn")
                nc.vector.tensor_mul(cn[:], gact[:, 128:256], c0_sb[:])
                nc.vector.tensor_add(cn[:], cn[:], ig[:])
                tc_sb = act.tile([B, 128], F32, tag="tc")
                nc.scalar.activation(tc_sb[:], cn[:], AFT.Tanh)
                hn = act.tile([B, 128], F32, tag="hn")
                nc.vector.tensor_mul(hn[:], gact[:, 256:384], tc_sb[:])

                if t + 1 < T:
                    trh = ps_tr.tile([128, 4, B], F32, tag="tr")
                    nc.tensor.transpose(trh[:, 0, :], hn[:], eye_sb[:])
                    hTo = act.tile([128, B], WDT, tag="hTo")
                    nc.vector.tensor_copy(hTo[:], trh[:, 0, :])

                    ag_in = dram.tile([128, B], WDT, tag="ag_in")
                    ag_out = dram.tile([UNITS, B], WDT, tag="ag_out")
                    nc.gpsimd.dma_start(ag_in[:], hTo[:])
                    nc.gpsimd.collective_compute(
                        "AllGather", mybir.AluOpType.bypass,
                        ins=[ag_in.opt()], outs=[ag_out.opt()],
                        replica_groups=[list(range(N_CORES))],
                    )
                    hT_next = [state.tile([128, 2, B], WDT, tag=f"hTp{p}",
                                          name=f"hTpn{p}") for p in range(4)]
                    ag_v = ag_out[:].rearrange("(c p) b -> p c b", p=128)
                    for p in range(4):
                        dma_engines[p].dma_start(
                            hT_next[p][:], ag_v[:, 2 * p:2 * p + 2, :])
                else:
                    hT_next = None

                # y output: runs inside the AllGather window
                y_ps = ps_y.tile([B, TOKEN], F32, tag="y")
                nc.tensor.matmul(y_ps[:], ones_r[:], bd2_r[:], start=True, stop=False)
                for k in range(KC):
                    nc.tensor.matmul(y_ps[:], a1T[:, k, :], wd2_r[:, k, :],
                                     start=False, stop=(k == KC - 1))
                y_sb = act.tile([B, TOKEN], F32, tag="y_sb")
                nc.scalar.activation(y_sb[:], y_ps[:], AFT.Copy)
                nc.scalar.dma_start(ys_t[:, t, :], y_sb[:])

                if t + 1 < T:
                    hTp = hT_next

    nc.compile()
    return nc


def _to_wdt(a, wdt_name):
    if wdt_name == "float16":
        return np.asarray(a, np.float16)
    return np.asarray(a, np.float32)


def make_in_maps(inputs: dict, wdt_name: str = WDT_NAME):
    s_0 = np.asarray(inputs["s_0"], np.float32)
    c_0 = np.asarray(inputs["c_0"], np.float32)
    Wd1 = np.asarray(inputs["Wd1"], np.float64)
    bd1 = np.asarray(inputs["bd1"], np.float64)
    Wd2 = np.asarray(inputs["Wd2"], np.float64)
    bd2 = np.asarray(inputs["bd2"], np.float64)
    Ws = {g: np.asarray(inputs[f"W_{g}"], np.float64) for g in "ifog"}
    bs = {g: np.asarray(inputs[f"b_{g}"], np.float64) for g in "ifog"}

    eye = np.eye(128, dtype=np.float32)
    in_maps = []
    for j in range(N_CORES):
        sl = slice(128 * j, 128 * (j + 1))
        wfu = np.concatenate([Wd2 @ Ws[g][:TOKEN, sl] for g in "ifog"], axis=1)
        wh = np.concatenate([Ws[g][TOKEN:, sl] for g in "ifog"], axis=1)
        bg = np.concatenate(
            [bs[g][sl] + bd2 @ Ws[g][:TOKEN, sl] for g in "ifog"])
        in_maps.append({
            "h0T": np.ascontiguousarray(_to_wdt(s_0.T.astype(np.float64), wdt_name)),
            "c0s": np.ascontiguousarray(c_0[:, sl]),
            "wd1": np.ascontiguousarray(_to_wdt(Wd1, wdt_name)),
            "wd2": np.ascontiguousarray(_to_wdt(Wd2, wdt_name)),
            "wfu": np.ascontiguousarray(_to_wdt(wfu, wdt_name)),
            "wh": np.ascontiguousarray(_to_wdt(wh, wdt_name)),
            "bd1row": np.ascontiguousarray(_to_wdt(bd1[None, :], wdt_name)),
            "bd2row": np.ascontiguousarray(_to_wdt(bd2[None, :], wdt_name)),
            "bgrow": np.ascontiguousarray(_to_wdt(bg[None, :], wdt_name)),
            "eye": eye,
        })
    return in_maps


def run(nc, in_maps, trace=False, **kw):
    return run_bass_kernel_spmd(nc, in_maps, core_ids=list(range(N_CORES)),
                                trace=trace, **kw)


_NC_CACHE = {}


def kernel(**inputs) -> np.ndarray:
    key = (T_FULL, WDT_NAME)
    if key not in _NC_CACHE:
        _NC_CACHE[key] = build(T_FULL, WDT_NAME)
    nc = _NC_CACHE[key]
    in_maps = make_in_maps(inputs, WDT_NAME)
    res = run(nc, in_maps)
    return np.asarray(res.results[0]["ys"], dtype=np.float32)



# revision 11
# speedup vs baseline: 1.0206x; 1.0206x over previous
"""DecoderLSTM Trainium2 kernel — 8-core tensor-parallel over gate output units.

Reference semantics (c_0 frozen by the original loop's bug):
    a1 = tanh(h @ Wd1 + bd1)                  # [B, U]
    y  = a1 @ Wd2 + bd2                       # [B, TOKEN] (the per-step output)
    xh = [y, h]
    i,f,o = sigmoid(xh @ W_*), g = tanh(xh @ W_g)
    c = f*c_0 + i*g ; h' = o*tanh(c)

Wd2 is folded into the gate weights on the host (Wfused = Wd2 @ W_*[:TOKEN],
bhat = b_* + bd2 @ W_*[:TOKEN]), so the gates contract over [a1, h] (K=2048)
and y moves off the recurrence's critical path.

Distribution: 8-way tensor-parallel over gate output units (128 units/gate/core,
slab order i|f|g|o). Each step ends with an exchange of the transposed h'
slices landing directly in h^T layout [128, 8slots, B].

Exchange transport (chosen at runtime by a small probe NEFF):
  - "rdma": direct SBUF->SBUF remote_dma_broadcast to the 7 XOR-relative
    peers; own slice is written locally into slot 0 by the PSUM->SBUF cast.
    Slot k on core j receives the slice of jax-core perm[j][k] (measured by
    the probe); the per-core weight row-blocks are permuted to match, so the
    program stays identical across cores. Receive gates (remote-sem waits) are
    injected POST-SCHEDULE: the single-core scheduling sim cannot model peer
    increments and would deadlock on them.
  - "cc": ncfw AllGather through DRAM (fallback; rank-ordered slots).

All matmuls: stationary = transposed activations [128, B], moving = weights
[128, N<=512] fp16 (full-rate), fp32 PSUM accumulation. Transposes in fp16.
"""
import numpy as np

import concourse.bacc as bacc
import concourse.tile as tile
import concourse.mybir as mybir
from concourse.bass_utils import run_bass_kernel_spmd
from concourse.instruction_name_ordered_set import InstructionNameOrderedSet


def _pin_after(inst, names):
    s = InstructionNameOrderedSet()
    for n in names:
        s.add(n)
    inst.ins.add_nosync_dependencies_from(s)

N_CORES = 8
B = 128
UNITS = 1024
TOKEN = 512
T_FULL = 256
KC = UNITS // 128
F32 = mybir.dt.float32
AFT = mybir.ActivationFunctionType

WDT_NAME = "float16"

RSEM_STEP = 14    # 7 remote senders x 2 per arriving transfer
N_JUNK = 20       # warm-keeper matmuls filling the collective window
LSEM_STEP = 112   # 7 sends x 16 on local send-complete


# ---------------------------------------------------------------------------
# probe: tiny NEFF that measures which jax-core's data lands in which slot
# ---------------------------------------------------------------------------

def build_probe():
    nc = bacc.Bacc("TRN2", target_bir_lowering=False, debug=False,
                   num_devices=N_CORES)
    P, F = 128, 16
    x_t = nc.dram_tensor("x", [P, F], F32, kind="ExternalInput").ap()
    out_t = nc.dram_tensor("out", [P, N_CORES, F], F32,
                           kind="ExternalOutput").ap()
    rsem = nc.alloc_semaphore(name="rdma_rsem")
    lsem = nc.alloc_semaphore(name="rdma_lsem")

    with tile.TileContext(nc) as tc:
        with tc.tile_pool(name="sb", bufs=1) as sb:
            x_sb = sb.tile([P, F], F32, tag="x")
            nc.sync.dma_start(x_sb[:], x_t[:])
            gat = sb.tile([P, N_CORES, F], F32, tag="gat")
            nc.vector.memset(gat[:], 0.0)
            cp = nc.vector.tensor_copy(gat[:, 0, :], x_sb[:])
            for k in range(1, N_CORES):
                rdests = [None] * N_CORES
                rdests[k] = (0, k)
                nc.gpsimd.remote_dma_broadcast(
                    gat[:, k, :], x_sb[:], rsem, lsem, rdests=rdests)
            nc.gpsimd.trigger_dma(count=None)
            w = nc.sync.nop(nofuse=True, hint="rdma_gate")
            _pin_after(w, [cp.ins.name])
            od = nc.sync.dma_start(out_t[:], gat[:])
            _pin_after(od, [w.ins.name])
    w._wait_ge(rsem, RSEM_STEP)
    nc.compile()
    return nc


def discover_topology():
    """Returns ("rdma", perm) with perm[j][k] = source jax-core of slot k on
    core j, or ("cc", rank-identity) if the remote-DMA path doesn't work."""
    ident = [[k for k in range(N_CORES)] for _ in range(N_CORES)]
    try:
        nc = build_probe()
        in_maps = []
        for j in range(N_CORES):
            x = np.zeros((128, 16), np.float32)
            x[:, :] = j * 1000 + np.arange(128)[:, None]
            in_maps.append({"x": x})
        res = run_bass_kernel_spmd(nc, in_maps, core_ids=list(range(N_CORES)))
        perm = []
        for j in range(N_CORES):
            out = np.asarray(res.results[j]["out"])
            row = []
            for k in range(N_CORES):
                src = int(round(out[0, k, 0] / 1000.0))
                ref = np.zeros((128, 16), np.float32)
                ref[:, :] = src * 1000 + np.arange(128)[:, None]
                if not (0 <= src < N_CORES) or not np.array_equal(out[:, k, :], ref):
                    raise RuntimeError(f"probe: garbled slot {k} on core {j}")
                row.append(src)
            if row[0] != j or sorted(row) != list(range(N_CORES)):
                raise RuntimeError(f"probe: invalid perm row {j}: {row}")
            perm.append(row)
        return "rdma", perm
    except Exception as e:  # noqa: BLE001
        import traceback
        print(f"rdma probe failed ({e}); falling back to ncfw collective")
        traceback.print_exc()
        return "cc", ident


# ---------------------------------------------------------------------------
# main kernel
# ---------------------------------------------------------------------------

def build(T: int, mode: str, wdt_name: str = WDT_NAME):
    WDT = getattr(mybir.dt, wdt_name)
    rdma = mode == "rdma"
    nc = bacc.Bacc("TRN2", target_bir_lowering=False, debug=False,
                   num_devices=N_CORES)

    def din(name, shape, dt=F32):
        return nc.dram_tensor(name, list(shape), dt, kind="ExternalInput").ap()

    h0T_t = din("h0T", (UNITS, B), WDT)
    c0s_t = din("c0s", (B, 128))
    wd1_t = din("wd1", (UNITS, UNITS), WDT)
    wd2_t = din("wd2", (UNITS, TOKEN), WDT)
    wfu_t = din("wfu", (UNITS, 512), WDT)
    wh_t = din("wh", (UNITS, 512), WDT)
    bd1_t = din("bd1row", (1, UNITS), WDT)
    bd2_t = din("bd2row", (1, TOKEN), WDT)
    bg_t = din("bgrow", (1, 512), WDT)
    eye_t = din("eye", (128, 128), WDT)

    ys_t = nc.dram_tensor("ys", [B, T, TOKEN], F32, kind="ExternalOutput").ap()

    if rdma:
        rsem = nc.alloc_semaphore(name="rdma_rsem")
        lsem = nc.alloc_semaphore(name="rdma_lsem")
    patches = []  # (BassInstruction, sem, value) applied post-schedule

    with tile.TileContext(nc) as tc:
        with tc.tile_pool(name="const", bufs=1) as const, \
             tc.tile_pool(name="state", bufs=1) as state, \
             tc.tile_pool(name="act", bufs=2) as act, \
             tc.tile_pool(name="ps_a1", bufs=2, space="PSUM") as ps_a1, \
             tc.tile_pool(name="ps_g", bufs=2, space="PSUM") as ps_g, \
             tc.tile_pool(name="ps_y", bufs=2, space="PSUM") as ps_y, \
             tc.tile_pool(name="ps_tr", bufs=2, space="PSUM") as ps_tr, \
             tc.tile_pool(name="dram", bufs=2, space="DRAM") as dram:

            def load_w(name, dram_ap, kdim, ndim):
                r = const.tile([128, kdim, ndim], WDT, tag=name, name=name)
                nc.sync.dma_start(r[:], dram_ap.rearrange("(c p) n -> p c n", p=128))
                return r

            wd1_r = load_w("wd1s", wd1_t, KC, UNITS)
            wd2_r = load_w("wd2s", wd2_t, KC, TOKEN)
            wfu_r = load_w("wfus", wfu_t, KC, 512)
            wh_r = load_w("whs", wh_t, KC, 512)

            def load_row(name, dram_ap, n):
                r = const.tile([1, n], WDT, tag=name, name=name)
                nc.sync.dma_start(r[:], dram_ap)
                return r

            bd1_r = load_row("bd1s", bd1_t, UNITS)
            bd2_r = load_row("bd2s", bd2_t, TOKEN)
            bg_r = load_row("bgs", bg_t, 512)

            ones_f = const.tile([1, 128], F32, tag="ones_f")
            nc.vector.memset(ones_f[:], 1.0)
            ones_r = const.tile([1, 128], WDT, tag="ones_r")
            nc.vector.tensor_copy(ones_r[:], ones_f[:])

            eye_sb = const.tile([128, 128], WDT, tag="eye")
            nc.sync.dma_start(eye_sb[:], eye_t[:])

            c0_sb = const.tile([B, 128], F32, tag="c0")
            nc.sync.dma_start(c0_sb[:], c0s_t[:])

            # h^T state: two fixed buffers (remote writes land here), 8 slots
            hTbuf = [state.tile([128, KC, B], WDT, tag=f"hTbuf{p}",
                                name=f"hTbuf{p}") for p in range(2)]
            h0_v = h0T_t.rearrange("(c p) b -> p c b", p=128)
            nc.sync.dma_start(hTbuf[0][:, 0:4, :], h0_v[:, 0:4, :])
            nc.scalar.dma_start(hTbuf[0][:, 4:8, :], h0_v[:, 4:8, :])

            a1h_ps = None
            g_psv = None
            slot0_done = False
            prev_trigger = None
            prev_trh = None
            win_mms = []

            for t in range(T):
                p, q = t % 2, (t + 1) % 2
                hT = hTbuf[p]

                # descriptor generation for this step's sends (fires at end)
                if rdma and t + 1 < T:
                    if t >= 2:
                        g = nc.gpsimd.nop(nofuse=True, hint=f"ringw{t}")
                        if prev_trigger is not None:
                            _pin_after(g, [prev_trigger.ins.name])
                        patches.append((g, lsem, LSEM_STEP * (t - 1)))
                    for k in range(1, N_CORES):
                        rdests = [None] * N_CORES
                        rdests[k] = (0, k)
                        pr = nc.gpsimd.remote_dma_broadcast(
                            hTbuf[q][:, k, :], hTbuf[q][:, 0, :],
                            rsem, lsem, rdests=rdests)
                        if prev_trigger is not None:
                            _pin_after(pr, [prev_trigger.ins.name])

                # receive gate for h(t): all 7 peer slices arrived
                if rdma and t >= 1:
                    w = nc.tensor.nop(nofuse=True, hint=f"hgate{t}")
                    pins = [m.ins.name for m in win_mms]
                    if prev_trh is not None:
                        pins.append(prev_trh.ins.name)
                    if pins:
                        _pin_after(w, pins)
                    patches.append((w, rsem, RSEM_STEP * t))

                if a1h_ps is None:  # t == 0 bootstrap biases
                    a1h_ps = [ps_a1.tile([B, 512], F32, tag="a1h",
                                         name=f"a1h_{t}_{i}") for i in range(2)]
                    g_psv = ps_g.tile([B, 512], F32, tag="g",
                                      name=f"g_{t}")
                    for half in range(2):
                        nc.tensor.matmul(a1h_ps[half][:], ones_r[:],
                                         bd1_r[:, 512 * half:512 * half + 512],
                                         start=True, stop=False)
                    nc.tensor.matmul(g_psv[:], ones_r[:], bg_r[:],
                                     start=True, stop=False)

                k0 = 1 if slot0_done else 0

                # Dense1 remaining chunks
                for half in range(2):
                    o0 = half * 512
                    for k in range(k0, KC):
                        nc.tensor.matmul(
                            a1h_ps[half][:], hT[:, k, :],
                            wd1_r[:, k, o0:o0 + 512],
                            start=False, stop=(k == KC - 1),
                        )
                # gates h-part remaining chunks
                for k in range(k0, KC):
                    nc.tensor.matmul(g_psv[:], hT[:, k, :], wh_r[:, k, :],
                                     start=False, stop=False)

                # tanh(a1) -> fp16, PE transpose -> a1T
                a1_sb = act.tile([B, UNITS], WDT, tag="a1_sb")
                a1T = act.tile([128, KC, B], WDT, tag="a1T")
                for grp in range(2):
                    o0 = grp * 512
                    nc.scalar.activation(a1_sb[:, o0:o0 + 512],
                                         a1h_ps[grp][:], AFT.Tanh)
                    tr = ps_tr.tile([128, 4, B], WDT, tag="tr")
                    for i in range(4):
                        c = grp * 4 + i
                        nc.tensor.transpose(tr[:, i, :],
                                            a1_sb[:, 128 * c:128 * (c + 1)],
                                            eye_sb[:])
                    nc.vector.tensor_copy(a1T[:, 4 * grp:4 * (grp + 1), :], tr[:])

                # gates a1-part
                for k in range(KC):
                    nc.tensor.matmul(g_psv[:], a1T[:, k, :], wfu_r[:, k, :],
                                     start=False, stop=(k == KC - 1))

                # tail (slab order i|f|g|o): c = f*c0 + i*g ; h' = o*tanh(c)
                gact = act.tile([B, 512], F32, tag="gact")
                nc.scalar.activation(gact[:, 0:256], g_psv[:, 0:256], AFT.Sigmoid)
                nc.scalar.activation(gact[:, 256:384], g_psv[:, 256:384], AFT.Tanh)
                nc.scalar.activation(gact[:, 384:512], g_psv[:, 384:512], AFT.Sigmoid)
                ig = act.tile([B, 128], F32, tag="ig")
                nc.vector.tensor_mul(ig[:], gact[:, 0:128], gact[:, 256:384])
                fc = act.tile([B, 128], F32, tag="fc")
                nc.gpsimd.tensor_mul(fc[:], gact[:, 128:256], c0_sb[:])
                cn = act.tile([B, 128], F32, tag="cn")
                nc.vector.tensor_add(cn[:], ig[:], fc[:])
                tc_sb = act.tile([B, 128], F32, tag="tc")
                nc.scalar.activation(tc_sb[:], cn[:], AFT.Tanh)
                hn16 = act.tile([B, 128], WDT, tag="hn")
                hnmul = nc.vector.tensor_mul(hn16[:], gact[:, 384:512], tc_sb[:])

                if t + 1 < T:
                    trh = ps_tr.tile([128, 4, B], WDT, tag="tr")
                    prev_trh = nc.tensor.transpose(trh[:, 0, :], hn16[:],
                                                   eye_sb[:])
                    if rdma:
                        if t >= 2:
                            v = nc.vector.nop(nofuse=True, hint=f"s0w{t}")
                            _pin_after(v, [hnmul.ins.name])
                            patches.append((v, lsem, LSEM_STEP * (t - 1)))
                        cslot = nc.vector.tensor_copy(hTbuf[q][:, 0, :],
                                                      trh[:, 0, :])
                        if t >= 2:
                            _pin_after(cslot, [v.ins.name])
                        prev_trigger = nc.gpsimd.trigger_dma(count=None)
                    else:
                        hTo = act.tile([128, B], WDT, tag="hTo")
                        nc.vector.tensor_copy(hTo[:], trh[:, 0, :])
                        ag_in = dram.tile([128, B], WDT, tag="ag_in")
                        ag_out = dram.tile([UNITS, B], WDT, tag="ag_out")
                        nc.gpsimd.dma_start(ag_in[:], hTo[:])
                        nc.gpsimd.collective_compute(
                            "AllGather", mybir.AluOpType.bypass,
                            ins=[ag_in.opt()], outs=[ag_out.opt()],
                            replica_groups=[list(range(N_CORES))],
                        )
                        ag_v = ag_out[:].rearrange("(c p) b -> p c b", p=128)
                        nc.sync.dma_start(hTbuf[q][:, 0:4, :], ag_v[:, 0:4, :])
                        nc.scalar.dma_start(hTbuf[q][:, 4:8, :], ag_v[:, 4:8, :])

                # ---- exchange window work ----
                # y output for this step
                y_ps = ps_y.tile([B, TOKEN], F32, tag="y")
                nc.tensor.matmul(y_ps[:], ones_r[:], bd2_r[:], start=True,
                                 stop=False)
                for k in range(KC):
                    nc.tensor.matmul(y_ps[:], a1T[:, k, :], wd2_r[:, k, :],
                                     start=False, stop=(k == KC - 1))
                y_sb = act.tile([B, TOKEN], F32, tag="y_sb")
                nc.scalar.activation(y_sb[:], y_ps[:], AFT.Copy)
                nc.scalar.dma_start(ys_t[:, t, :], y_sb[:])

                if t + 1 < T:
                    # next step's biases + (rdma) own-slice chunks
                    a1h_ps = [ps_a1.tile([B, 512], F32, tag="a1h",
                                         name=f"a1hn_{t}_{i}") for i in range(2)]
                    g_psv = ps_g.tile([B, 512], F32, tag="g",
                                      name=f"gn_{t}")
                    for half in range(2):
                        nc.tensor.matmul(a1h_ps[half][:], ones_r[:],
                                         bd1_r[:, 512 * half:512 * half + 512],
                                         start=True, stop=False)
                    nc.tensor.matmul(g_psv[:], ones_r[:], bg_r[:],
                                     start=True, stop=False)
                    if not rdma:
                        for _ in range(N_JUNK):
                            jp = ps_y.tile([B, 512], F32, tag="y",
                                           name=f"junk_{t}_{_}")
                            nc.tensor.matmul(jp[:], ones_r[:],
                                             bd1_r[:, 0:512],
                                             start=True, stop=True)
                    if rdma:
                        win_mms = []
                        for half in range(2):
                            o0 = half * 512
                            win_mms.append(nc.tensor.matmul(
                                a1h_ps[half][:], hTbuf[q][:, 0, :],
                                wd1_r[:, 0, o0:o0 + 512],
                                start=False, stop=False,
                            ))
                        win_mms.append(nc.tensor.matmul(
                            g_psv[:], hTbuf[q][:, 0, :],
                            wh_r[:, 0, :], start=False, stop=False))
                        slot0_done = True

    for inst, sem, val in patches:
        inst._wait_ge(sem, val)

    nc.compile()
    return nc


# ---------------------------------------------------------------------------
# host-side data prep
# ---------------------------------------------------------------------------

def _to_wdt(a, wdt_name):
    if wdt_name == "float16":
        return np.asarray(a, np.float16)
    return np.asarray(a, np.float32)


GATE_ORDER = "ifgo"  # slab order (o last so it can overlap the DVE chain)


def make_in_maps(inputs: dict, wdt_name: str = WDT_NAME, perm=None):
    if perm is None:
        perm = _STATE.get("perm")
    if perm is None:
        perm = [[k for k in range(N_CORES)] for _ in range(N_CORES)]
    s_0 = np.asarray(inputs["s_0"], np.float32)
    c_0 = np.asarray(inputs["c_0"], np.float32)
    Wd1 = np.asarray(inputs["Wd1"], np.float64)
    bd1 = np.asarray(inputs["bd1"], np.float64)
    Wd2 = np.asarray(inputs["Wd2"], np.float64)
    bd2 = np.asarray(inputs["bd2"], np.float64)
    Ws = {g: np.asarray(inputs[f"W_{g}"], np.float64) for g in "ifog"}
    bs = {g: np.asarray(inputs[f"b_{g}"], np.float64) for g in "ifog"}

    eye = np.eye(128, dtype=np.float32)
    s0T = s_0.T.astype(np.float64)  # [UNITS, B]
    in_maps = []
    for j in range(N_CORES):
        sl = slice(128 * j, 128 * (j + 1))
        row = perm[j]
        rp = np.concatenate([np.arange(128 * r, 128 * (r + 1)) for r in row])
        wfu = np.concatenate(
            [Wd2 @ Ws[g][:TOKEN, sl] for g in GATE_ORDER], axis=1)
        wh = np.concatenate(
            [Ws[g][TOKEN:, sl][rp, :] for g in GATE_ORDER], axis=1)
        bg = np.concatenate(
            [bs[g][sl] + bd2 @ Ws[g][:TOKEN, sl] for g in GATE_ORDER])
        in_maps.append({
            "h0T": np.ascontiguousarray(_to_wdt(s0T[rp, :], wdt_name)),
            "c0s": np.ascontiguousarray(c_0[:, sl]),
            "wd1": np.ascontiguousarray(_to_wdt(Wd1[rp, :], wdt_name)),
            "wd2": np.ascontiguousarray(_to_wdt(Wd2, wdt_name)),
            "wfu": np.ascontiguousarray(_to_wdt(wfu, wdt_name)),
            "wh": np.ascontiguousarray(_to_wdt(wh, wdt_name)),
            "bd1row": np.ascontiguousarray(_to_wdt(bd1[None, :], wdt_name)),
            "bd2row": np.ascontiguousarray(_to_wdt(bd2[None, :], wdt_name)),
            "bgrow": np.ascontiguousarray(_to_wdt(bg[None, :], wdt_name)),
            "eye": _to_wdt(eye, wdt_name),
        })
    return in_maps


def run(nc, in_maps, trace=False, **kw):
    return run_bass_kernel_spmd(nc, in_maps, core_ids=list(range(N_CORES)),
                                trace=trace, **kw)


_NC_CACHE = {}
_STATE = {"mode": None, "perm": None}


FORCE_MODE = "cc"  # rdma: remote-sem fires before data arrival on this runtime


def _ensure_topology():
    if _STATE["mode"] is None:
        if FORCE_MODE:
            _STATE["mode"] = FORCE_MODE
            _STATE["perm"] = [[k for k in range(N_CORES)]
                              for _ in range(N_CORES)]
        else:
            mode, perm = discover_topology()
            _STATE["mode"] = mode
            _STATE["perm"] = perm
    return _STATE["mode"], _STATE["perm"]


def kernel(**inputs) -> np.ndarray:
    mode, perm = _ensure_topology()
    key = (T_FULL, WDT_NAME, mode)
    if key not in _NC_CACHE:
        _NC_CACHE[key] = build(T_FULL, mode, WDT_NAME)
    nc = _NC_CACHE[key]
    in_maps = make_in_maps(inputs, WDT_NAME, perm)
    res = run(nc, in_maps)
    out = np.asarray(res.results[0]["ys"], dtype=np.float32)
    if mode == "rdma":
        # sanity: rdma garble would poison everything; cheap NaN check
        if not np.isfinite(out).all():
            raise RuntimeError("rdma kernel produced non-finite output")
    return out
